# revision 1
# baseline (speedup 1.0000x reference)
"""Trainium2 Bass kernel for nn_MultiHeadAttention_KT (causal linear attention).

Math (per batch b):
  q' = leaky((q*qm) @ Wq + bq); k' = leaky((k*km) @ Wk + bk); v' = (v*vm) @ Wv
  per head h (DEPTH=64):   S_t = sum_{s<=t} k_s v_s^T ; z_t = sum_{s<=t} k_s
                           attn_t = (q_t @ S_t) / (q_t . z_t)
  out = concat_heads(attn) @ Wo + bo

Sharding: 8 cores = 2 batches x 4 head-groups (4 heads / 256 cols each).
Host transposes inputs (xq = (q*qm)^T etc.) so the contraction dim lands on
SBUF partitions; host sums the 4 partial output projections per batch.

Chunked linear attention on device (chunk C=128, all matmuls on PE):
  AT   = K Q^T (per chunk, [s,t] layout)      masked with triu (s<=t)
  num  = ATm^T V_aug + Q S_aug                (V_aug = [V | 1], S_aug = [S | z])
  attn = num[:, :64] * (1/num[:, 64])
  S_aug += K_chunk^T V_aug                    (delta matmul + DVE add)
"""

import os
import sys

sys.path.insert(0, "/opt/trn_rl_repo")

import numpy as np

B, S, D, H = 2, 2048, 1024, 16
DEPTH = 64
N_CORES = 8
HPC = 4                 # heads per core
JS = HPC * DEPTH        # 256 projected columns per core
C = 128                 # attention chunk length
NCH = S // C            # 16 chunks
IB = D // 128           # 8 contraction blocks
SCH = 256               # projection s-chunk
NSC = S // SCH          # 8 projection chunks
JAUG = DEPTH + 1        # 65 (V augmented with ones column)

# Matmul operand dtype knob: "f32" (safe) or "f32r" (4x faster at N>=256).
MM_DTYPE = os.environ.get("KT_MM_DTYPE", "f32r")
TRACE = False           # set True from test harness to capture NTFF profile
TRACE_CORES = None
LAST_RESULTS = None     # BassKernelResults of the last kernel() call

_PROG = None


def _build():
    import concourse.bacc as bacc
    import concourse.mybir as mybir
    import concourse.tile as tile

    dt = mybir.dt
    f32 = dt.float32
    mmdt = {"f32": dt.float32, "f32r": dt.float32r}[MM_DTYPE]
    AF = mybir.ActivationFunctionType
    Alu = mybir.AluOpType

    nc = bacc.Bacc("TRN2", target_bir_lowering=False, debug=False,
                   num_devices=N_CORES)

    xq = nc.dram_tensor("xq", [D, S], f32, kind="ExternalInput").ap()
    xk = nc.dram_tensor("xk", [D, S], f32, kind="ExternalInput").ap()
    xv = nc.dram_tensor("xv", [D, S], f32, kind="ExternalInput").ap()
    wq = nc.dram_tensor("wq", [D, JS], f32, kind="ExternalInput").ap()
    wk = nc.dram_tensor("wk", [D, JS], f32, kind="ExternalInput").ap()
    wv = nc.dram_tensor("wv", [D, JS], f32, kind="ExternalInput").ap()
    wo = nc.dram_tensor("wo", [JS, D], f32, kind="ExternalInput").ap()
    bqd = nc.dram_tensor("bq", [2, 128], f32, kind="ExternalInput").ap()
    bkd = nc.dram_tensor("bk", [2, 128], f32, kind="ExternalInput").ap()
    triu = nc.dram_tensor("triu", [128, 128], f32, kind="ExternalInput").ap()
    ident = nc.dram_tensor("ident", [128, 128], f32, kind="ExternalInput").ap()
    ident2 = nc.dram_tensor("ident2", [128, 64], f32, kind="ExternalInput").ap()
    po = nc.dram_tensor("po", [D, S], f32, kind="ExternalOutput").ap()
    debug = os.environ.get("KT_DEBUG") == "1"
    if debug:
        qTo = nc.dram_tensor("qTo", [2, 128, S], f32, kind="ExternalOutput").ap()
        kTo = nc.dram_tensor("kTo", [2, 128, S], f32, kind="ExternalOutput").ap()
        vauo = nc.dram_tensor("vauo", [NCH, 128, HPC * JAUG], f32,
                              kind="ExternalOutput").ap()
        aTo = nc.dram_tensor("aTo", [2, 128, S], f32, kind="ExternalOutput").ap()

    def mm(out, lhsT, rhs, **kw):
        nc.tensor.matmul(out, lhsT, rhs, **kw)

    # float32r operands must be explicitly rounded by their producer:
    # DRAM loads go through a casting SWDGE DMA, on-chip producers write
    # mmdt-typed tiles. ldma = load-with-cast engine choice.
    ldma = nc.sync.dma_start if mmdt == f32 else nc.gpsimd.dma_start

    with tile.TileContext(nc) as tc:
        with (
            tc.tile_pool(name="persist", bufs=1) as pp,
            tc.tile_pool(name="xin", bufs=2) as xpool,
            tc.tile_pool(name="work", bufs=3) as wk_pool,
            tc.tile_pool(name="outp", bufs=3) as opool,
            tc.tile_pool(name="psA", bufs=4, space="PSUM") as psA,
            tc.tile_pool(name="psB", bufs=4, space="PSUM") as psB,
        ):
            # ---- Phase 0: weights + constants -------------------------------
            wq_sb = pp.tile([128, IB, JS], mmdt, tag="wq", name="wq_sb")
            wk_sb = pp.tile([128, IB, JS], mmdt, tag="wk", name="wk_sb")
            wv_sb = pp.tile([128, IB, JS], mmdt, tag="wv", name="wv_sb")
            wo_sb = pp.tile([128, 2, D], mmdt, tag="wo", name="wo_sb")
            ldma(wq_sb[:], wq.rearrange("(ib p) j -> p ib j", p=128))
            ldma(wk_sb[:], wk.rearrange("(ib p) j -> p ib j", p=128))
            ldma(wv_sb[:], wv.rearrange("(ib p) j -> p ib j", p=128))
            ldma(wo_sb[:], wo.rearrange("(jb p) o -> p jb o", p=128))
            bq_sb = pp.tile([128, 2], f32, tag="bq", name="bq_sb")
            bk_sb = pp.tile([128, 2], f32, tag="bk", name="bk_sb")
            nc.sync.dma_start(bq_sb[:], bqd.rearrange("jb p -> p jb"))
            nc.sync.dma_start(bk_sb[:], bkd.rearrange("jb p -> p jb"))
            triu_sb = pp.tile([128, 128], f32, tag="triu", name="triu_sb")
            ident_sb = pp.tile([128, 128], f32, tag="ident", name="ident_sb")
            ident2_sb = pp.tile([128, 64], f32, tag="ident2", name="ident2_sb")
            nc.sync.dma_start(triu_sb[:], triu)
            nc.sync.dma_start(ident_sb[:], ident)
            nc.sync.dma_start(ident2_sb[:], ident2)

            qT_sb = [pp.tile([128, S], f32, tag=f"qT{jb}", name=f"qT{jb}") for jb in range(2)]
            kT_sb = [pp.tile([128, S], f32, tag=f"kT{jb}", name=f"kT{jb}") for jb in range(2)]
            aT_sb = [pp.tile([128, S], mmdt, tag=f"aT{jb}", name=f"aT{jb}") for jb in range(2)]
            vaug_sb = [pp.tile([128, HPC * JAUG], f32, tag=f"vaug{i}", name=f"vaug{i}")
                       for i in range(NCH)]
            # two heads per tile: head h at partitions (h%2)*64 .. +64
            saug_sb = [pp.tile([128, JAUG], f32, tag=f"saug{jb}", name=f"saug{jb}")
                       for jb in range(2)]

            xq_r = xq.rearrange("(ib p) s -> p ib s", p=128)
            xk_r = xk.rearrange("(ib p) s -> p ib s", p=128)
            xv_r = xv.rearrange("(ib p) s -> p ib s", p=128)

            # ---- Phase 1: projections --------------------------------------
            for sc in range(NSC):
                s0 = sc * SCH
                xq_t = xpool.tile([128, IB, SCH], mmdt, tag="xq")
                xk_t = xpool.tile([128, IB, SCH], mmdt, tag="xk")
                xv_t = xpool.tile([128, IB, SCH], mmdt, tag="xv")
                ldma(xq_t[:], xq_r[:, :, s0:s0 + SCH])
                ldma(xk_t[:], xk_r[:, :, s0:s0 + SCH])
                ldma(xv_t[:], xv_r[:, :, s0:s0 + SCH])

                # q'/k' transposed: psum [128 j, SCH s]
                for name, w_sb, b_sb, dst in (
                    ("q", wq_sb, bq_sb, qT_sb),
                    ("k", wk_sb, bk_sb, kT_sb),
                ):
                    for jb in range(2):
                        ps = psA.tile([128, SCH], f32, tag="A")
                        for ib in range(IB):
                            mm(ps[:], w_sb[:, ib, jb * 128:(jb + 1) * 128],
                               xq_t[:, ib, :] if name == "q" else xk_t[:, ib, :],
                               start=(ib == 0), stop=(ib == IB - 1))
                        nc.scalar.activation(
                            dst[jb][:, s0:s0 + SCH], ps[:], AF.Prelu,
                            bias=b_sb[:, jb:jb + 1], scale=1.0, alpha=0.1)

                # v' natural: psum [128 s, JS], augmented store
                for ss in range(SCH // 128):
                    ps = psA.tile([128, JS], f32, tag="A")
                    for ib in range(IB):
                        mm(ps[:], xv_t[:, ib, ss * 128:(ss + 1) * 128],
                           wv_sb[:, ib, :],
                           start=(ib == 0), stop=(ib == IB - 1))
                    vt = vaug_sb[sc * 2 + ss]
                    vt_r = vt[:].rearrange("p (h e) -> p h e", h=HPC)
                    nc.scalar.activation(
                        vt_r[:, :, 0:DEPTH],
                        ps[:].rearrange("p (h e) -> p h e", h=HPC), AF.Copy)
                    nc.vector.memset(vt_r[:, :, DEPTH:JAUG], 1.0)

            # ---- Phase 2: chunked causal linear attention ------------------
            for ci in range(NCH):
                scol = ci * C
                for jb in range(2):
                    attn2 = wk_pool.tile([128, 2 * DEPTH], f32, tag="attn2")
                    for hh in range(2):
                        h = jb * 2 + hh
                        jo = hh * DEPTH
                        kT_v = kT_sb[jb][jo:jo + DEPTH, scol:scol + C]
                        qT_v = qT_sb[jb][jo:jo + DEPTH, scol:scol + C]
                        vt = vaug_sb[ci][:, h * JAUG:(h + 1) * JAUG]
                        saug_v = saug_sb[jb][jo:jo + DEPTH, :]

                        # K natural [s, d] via PE transpose
                        knat_ps = psB.tile([128, DEPTH], f32, tag="B")
                        nc.tensor.transpose(knat_ps[:], kT_v,
                                            ident2_sb[jo:jo + DEPTH, :])
                        knat = wk_pool.tile([128, DEPTH], f32, tag="knat")
                        nc.vector.tensor_copy(knat[:], knat_ps[:])

                        # AT = K Q^T  [s, t]; mask s<=t
                        at_ps = psA.tile([128, C], f32, tag="A")
                        mm(at_ps[:], kT_v, qT_v, start=True, stop=True)
                        atm = wk_pool.tile([128, C], f32, tag="atm")
                        nc.vector.tensor_tensor(atm[:], at_ps[:], triu_sb[:],
                                                op=Alu.mult)

                        # num[t, 0:64] + den[t, 64]
                        num_ps = psB.tile([128, JAUG], f32, tag="B")
                        mm(num_ps[:], atm[:], vt, start=True, stop=(ci == 0))
                        if ci > 0:
                            mm(num_ps[:], qT_v, saug_v,
                               start=False, stop=True)

                        recip = wk_pool.tile([128, 1], f32, tag="recip")
                        nc.vector.reciprocal(recip[:], num_ps[:, DEPTH:JAUG])
                        nc.scalar.activation(attn2[:, jo:jo + DEPTH],
                                             num_ps[:, 0:DEPTH], AF.Copy,
                                             scale=recip[:])

                        # state update S_aug += K^T V_aug
                        if ci < NCH - 1:
                            d_ps = psB.tile([128, JAUG], f32, tag="B")
                            mm(d_ps[jo:jo + DEPTH, :], knat[:], vt,
                               start=True, stop=True)
                            if ci == 0:
                                nc.vector.tensor_copy(saug_v,
                                                      d_ps[jo:jo + DEPTH, :])
                            else:
                                nc.vector.tensor_add(saug_v, saug_v,
                                                     d_ps[jo:jo + DEPTH, :])

                    # both heads' attn^T in one shot -> [128, C] at base 0
                    at2_ps = psB.tile([128, C], f32, tag="B")
                    nc.tensor.transpose(at2_ps[:], attn2[:], ident_sb[:])
                    nc.scalar.activation(aT_sb[jb][:, scol:scol + C],
                                         at2_ps[:], AF.Copy)

            if debug:
                for jb in range(2):
                    nc.sync.dma_start(qTo[jb], qT_sb[jb][:])
                    nc.sync.dma_start(kTo[jb], kT_sb[jb][:])
                    nc.sync.dma_start(aTo[jb], aT_sb[jb][:].bitcast(f32))
                for i in range(NCH):
                    nc.sync.dma_start(vauo[i], vaug_sb[i][:])

            # ---- Phase 3: output projection (partial: this core's heads) ---
            po_r = po.rearrange("(ob p) s -> ob p s", p=128)
            OSC = min(512, S)
            for ob in range(D // 128):
                for sq in range(S // OSC):
                    ps = psA.tile([128, OSC], f32, tag="A")
                    for jb in range(2):
                        mm(ps[:], wo_sb[:, jb, ob * 128:(ob + 1) * 128],
                           aT_sb[jb][:, sq * OSC:(sq + 1) * OSC],
                           start=(jb == 0), stop=(jb == 1))
                    ot = opool.tile([128, OSC], f32, tag="ot")
                    if (ob + sq) % 2 == 0:
                        nc.scalar.activation(ot[:], ps[:], AF.Copy)
                    else:
                        nc.vector.tensor_copy(ot[:], ps[:])
                    nc.sync.dma_start(po_r[ob, :, sq * OSC:(sq + 1) * OSC],
                                      ot[:])

    nc.compile()
    return nc


def _get_prog():
    global _PROG
    if _PROG is None:
        _PROG = _build()
    return _PROG


def kernel(q, k, v, query_mask, key_mask, value_mask,
           Wq, bq, Wk, bk, Wv, bv, Wo, bo):
    global LAST_RESULTS
    from concourse import bass_utils

    q = np.asarray(q, np.float32)
    k = np.asarray(k, np.float32)
    v = np.asarray(v, np.float32)
    qm = q * np.asarray(query_mask, np.float32)
    km = k * np.asarray(key_mask, np.float32)
    vm = v * np.asarray(value_mask, np.float32)
    Wq = np.asarray(Wq, np.float32)
    Wk = np.asarray(Wk, np.float32)
    Wv = np.asarray(Wv, np.float32)
    Wo = np.asarray(Wo, np.float32)
    bq = np.asarray(bq, np.float32)
    bk = np.asarray(bk, np.float32)
    bv = np.asarray(bv, np.float32)
    bo = np.asarray(bo, np.float32)
    assert not np.any(bv), "kernel assumes bv == 0 (true for this problem)"

    nc = _get_prog()

    triu = np.triu(np.ones((128, 128), np.float32))
    ident = np.eye(128, dtype=np.float32)
    ident2 = np.tile(np.eye(64, dtype=np.float32), (2, 1))
    xqs = [np.ascontiguousarray(qm[b].T) for b in range(B)]
    xks = [np.ascontiguousarray(km[b].T) for b in range(B)]
    xvs = [np.ascontiguousarray(vm[b].T) for b in range(B)]

    in_maps = []
    for c in range(N_CORES):
        b, g = divmod(c, HPC)
        js = slice(g * JS, (g + 1) * JS)
        in_maps.append({
            "xq": xqs[b], "xk": xks[b], "xv": xvs[b],
            "wq": np.ascontiguousarray(Wq[:, js]),
            "wk": np.ascontiguousarray(Wk[:, js]),
            "wv": np.ascontiguousarray(Wv[:, js]),
            "wo": np.ascontiguousarray(Wo[js, :]),
            "bq": np.ascontiguousarray(bq[js].reshape(2, 128)),
            "bk": np.ascontiguousarray(bk[js].reshape(2, 128)),
            "triu": triu, "ident": ident, "ident2": ident2,
        })

    res = bass_utils.run_bass_kernel_spmd(
        nc, in_maps, core_ids=list(range(N_CORES)),
        trace=TRACE, trace_cores=TRACE_CORES)
    LAST_RESULTS = res

    out = np.zeros((B, S, D), np.float32)
    for c in range(N_CORES):
        out[c // HPC] += res.results[c]["po"].T
    out += bo
    return out



# revision 2
# speedup vs baseline: 1.3862x; 1.3862x over previous
"""Trainium2 Bass kernel for nn_MultiHeadAttention_KT (causal linear attention).

Math (per batch b):
  q' = leaky((q*qm) @ Wq + bq); k' = leaky((k*km) @ Wk + bk); v' = (v*vm) @ Wv
  per head h (DEPTH=64):   S_t = sum_{s<=t} k_s v_s^T ; z_t = sum_{s<=t} k_s
                           attn_t = (q_t @ S_t) / (q_t . z_t)
  out = concat_heads(attn) @ Wo + bo

Sharding: 8 cores = 2 batches x 4 head-groups (4 heads / 256 cols each).
Host transposes inputs (xq = (q*qm)^T etc.), casts everything to bf16,
and sums the 4 partial output projections per batch.

All matmuls run in bf16 (1 cycle/row on PE regardless of free-dim size;
f32 accumulate in PSUM). Transposes (K chunk -> natural layout, attn ->
attn^T) run on the DMA xbar (dma_start_transpose), not the PE.

Chunked linear attention on device (chunk C=128, all matmuls on PE):
  AT   = K Q^T (per chunk, [s,t] layout)      masked with triu (s<=t)
  num  = ATm^T V_aug + Q S_aug                (V_aug = [V | 1], S_aug = [S | z])
  attn = num[:, :64] * (1/num[:, 64])
  S_aug += K_chunk^T V_aug    (delta matmul; f32 master state + bf16 copy)
"""

import os
import sys

sys.path.insert(0, "/opt/trn_rl_repo")

import numpy as np
import ml_dtypes

BF16 = ml_dtypes.bfloat16

B, S, D, H = 2, 2048, 1024, 16
DEPTH = 64
N_CORES = 8
HPC = 4                 # heads per core
JS = HPC * DEPTH        # 256 projected columns per core
C = 128                 # attention chunk length
NCH = S // C            # 16 chunks
IB = D // 128           # 8 contraction blocks
SCH = 512               # projection s-chunk
NSC = S // SCH          # 4 projection chunks
JAUG = DEPTH + 1        # 65 (V augmented with ones column)

TRACE = False           # set True from test harness to capture NTFF profile
TRACE_CORES = None
LAST_RESULTS = None     # BassKernelResults of the last kernel() call

_PROG = None


def _build():
    import concourse.bacc as bacc
    import concourse.mybir as mybir
    import concourse.tile as tile

    dt = mybir.dt
    f32 = dt.float32
    bf16 = dt.bfloat16
    AF = mybir.ActivationFunctionType
    Alu = mybir.AluOpType

    nc = bacc.Bacc("TRN2", target_bir_lowering=False, debug=False,
                   num_devices=N_CORES)

    xq = nc.dram_tensor("xq", [D, S], bf16, kind="ExternalInput").ap()
    xk = nc.dram_tensor("xk", [D, S], bf16, kind="ExternalInput").ap()
    xv = nc.dram_tensor("xv", [D, S], bf16, kind="ExternalInput").ap()
    wq = nc.dram_tensor("wq", [D, JS], bf16, kind="ExternalInput").ap()
    wk = nc.dram_tensor("wk", [D, JS], bf16, kind="ExternalInput").ap()
    wv = nc.dram_tensor("wv", [D, JS], bf16, kind="ExternalInput").ap()
    wo = nc.dram_tensor("wo", [JS, D], bf16, kind="ExternalInput").ap()
    bqd = nc.dram_tensor("bq", [2, 128], f32, kind="ExternalInput").ap()
    bkd = nc.dram_tensor("bk", [2, 128], f32, kind="ExternalInput").ap()
    triu = nc.dram_tensor("triu", [128, 128], f32, kind="ExternalInput").ap()
    po = nc.dram_tensor("po", [D, S], bf16, kind="ExternalOutput").ap()

    def mm(out, lhsT, rhs, **kw):
        nc.tensor.matmul(out, lhsT, rhs, **kw)

    with tile.TileContext(nc) as tc:
        with (
            tc.tile_pool(name="persist", bufs=1) as pp,
            tc.tile_pool(name="xin", bufs=3) as xpool,
            tc.tile_pool(name="work", bufs=4) as wk_pool,
            tc.tile_pool(name="outp", bufs=3) as opool,
            tc.tile_pool(name="psA", bufs=4, space="PSUM") as psA,
            tc.tile_pool(name="psB", bufs=4, space="PSUM") as psB,
        ):
            # ---- Phase 0: weights + constants -------------------------------
            wq_sb = pp.tile([128, IB, JS], bf16, tag="wq", name="wq_sb")
            wk_sb = pp.tile([128, IB, JS], bf16, tag="wk", name="wk_sb")
            wv_sb = pp.tile([128, IB, JS], bf16, tag="wv", name="wv_sb")
            wo_sb = pp.tile([128, 2, D], bf16, tag="wo", name="wo_sb")
            nc.sync.dma_start(wq_sb[:], wq.rearrange("(ib p) j -> p ib j", p=128))
            nc.sync.dma_start(wk_sb[:], wk.rearrange("(ib p) j -> p ib j", p=128))
            nc.sync.dma_start(wv_sb[:], wv.rearrange("(ib p) j -> p ib j", p=128))
            nc.sync.dma_start(wo_sb[:], wo.rearrange("(jb p) o -> p jb o", p=128))
            bq_sb = pp.tile([128, 2], f32, tag="bq", name="bq_sb")
            bk_sb = pp.tile([128, 2], f32, tag="bk", name="bk_sb")
            nc.sync.dma_start(bq_sb[:], bqd.rearrange("jb p -> p jb"))
            nc.sync.dma_start(bk_sb[:], bkd.rearrange("jb p -> p jb"))
            triu_sb = pp.tile([128, 128], f32, tag="triu", name="triu_sb")
            nc.sync.dma_start(triu_sb[:], triu)

            qT_sb = [pp.tile([128, S], bf16, tag=f"qT{jb}", name=f"qT{jb}") for jb in range(2)]
            kT_sb = [pp.tile([128, S], bf16, tag=f"kT{jb}", name=f"kT{jb}") for jb in range(2)]
            aT_sb = [pp.tile([128, S], bf16, tag=f"aT{jb}", name=f"aT{jb}") for jb in range(2)]
            # K in natural layout per (chunk, head): [128 s, 64 d]
            knat_sb = pp.tile([128, NCH, HPC, DEPTH], bf16, tag="knat",
                              name="knat_sb")
            vaug_sb = [pp.tile([128, HPC * JAUG], bf16, tag=f"vaug{i}",
                               name=f"vaug{i}")
                       for i in range(NCH)]
            # two heads per tile: head h at partitions (h%2)*64 .. +64
            saug_sb = [pp.tile([128, JAUG], f32, tag=f"saug{jb}", name=f"saug{jb}")
                       for jb in range(2)]
            saug_bf = [pp.tile([128, JAUG], bf16, tag=f"saugb{jb}",
                               name=f"saugb{jb}")
                       for jb in range(2)]

            xq_r = xq.rearrange("(ib p) s -> p ib s", p=128)
            xk_r = xk.rearrange("(ib p) s -> p ib s", p=128)
            xv_r = xv.rearrange("(ib p) s -> p ib s", p=128)

            # ---- Phase 1: projections --------------------------------------
            for sc in range(NSC):
                s0 = sc * SCH
                xq_t = xpool.tile([128, IB, SCH], bf16, tag="xq")
                xk_t = xpool.tile([128, IB, SCH], bf16, tag="xk")
                xv_t = xpool.tile([128, IB, SCH], bf16, tag="xv")
                nc.sync.dma_start(xq_t[:], xq_r[:, :, s0:s0 + SCH])
                nc.sync.dma_start(xk_t[:], xk_r[:, :, s0:s0 + SCH])
                nc.sync.dma_start(xv_t[:], xv_r[:, :, s0:s0 + SCH])

                # q'/k' transposed: psum [128 j, SCH s]
                for name, w_sb, b_sb, dst in (
                    ("q", wq_sb, bq_sb, qT_sb),
                    ("k", wk_sb, bk_sb, kT_sb),
                ):
                    for jb in range(2):
                        ps = psA.tile([128, SCH], f32, tag="A")
                        for ib in range(IB):
                            mm(ps[:], w_sb[:, ib, jb * 128:(jb + 1) * 128],
                               xq_t[:, ib, :] if name == "q" else xk_t[:, ib, :],
                               start=(ib == 0), stop=(ib == IB - 1))
                        nc.scalar.activation(
                            dst[jb][:, s0:s0 + SCH], ps[:], AF.Prelu,
                            bias=b_sb[:, jb:jb + 1], scale=1.0, alpha=0.1)

                # v' natural: psum [128 s, JS], augmented store
                for ss in range(SCH // 128):
                    ps = psA.tile([128, JS], f32, tag="A")
                    for ib in range(IB):
                        mm(ps[:], xv_t[:, ib, ss * 128:(ss + 1) * 128],
                           wv_sb[:, ib, :],
                           start=(ib == 0), stop=(ib == IB - 1))
                    vt = vaug_sb[sc * (SCH // 128) + ss]
                    vt_r = vt[:].rearrange("p (h e) -> p h e", h=HPC)
                    nc.scalar.activation(
                        vt_r[:, :, 0:DEPTH],
                        ps[:].rearrange("p (h e) -> p h e", h=HPC), AF.Copy)
                    nc.vector.memset(vt_r[:, :, DEPTH:JAUG], 1.0)

            # K natural layout via DMA-xbar transpose (off the PE)
            for ci in range(NCH):
                for h in range(HPC):
                    jb, hh = divmod(h, 2)
                    jo = hh * DEPTH
                    nc.sync.dma_start_transpose(
                        knat_sb[:, ci, h, :],
                        kT_sb[jb][jo:jo + DEPTH, ci * C:(ci + 1) * C])

            # ---- Phase 2: chunked causal linear attention ------------------
            for ci in range(NCH):
                scol = ci * C
                for jb in range(2):
                    attn2 = wk_pool.tile([128, 2 * DEPTH], bf16, tag="attn2")
                    d_ps = psB.tile([128, JAUG], f32, tag="B")
                    for hh in range(2):
                        h = jb * 2 + hh
                        jo = hh * DEPTH
                        kT_v = kT_sb[jb][jo:jo + DEPTH, scol:scol + C]
                        qT_v = qT_sb[jb][jo:jo + DEPTH, scol:scol + C]
                        vt = vaug_sb[ci][:, h * JAUG:(h + 1) * JAUG]
                        saug_v = saug_sb[jb][jo:jo + DEPTH, :]

                        # AT = K Q^T  [s, t]; mask s<=t
                        at_ps = psA.tile([128, C], f32, tag="A")
                        mm(at_ps[:], kT_v, qT_v, start=True, stop=True)
                        atm = wk_pool.tile([128, C], bf16, tag="atm")
                        nc.vector.tensor_tensor(atm[:], at_ps[:], triu_sb[:],
                                                op=Alu.mult)

                        # num[t, 0:64] + den[t, 64]
                        num_ps = psB.tile([128, JAUG], f32, tag="B")
                        mm(num_ps[:], atm[:], vt, start=True, stop=(ci == 0))
                        if ci > 0:
                            mm(num_ps[:], qT_v,
                               saug_bf[jb][jo:jo + DEPTH, :],
                               start=False, stop=True)

                        recip = wk_pool.tile([128, 1], f32, tag="recip")
                        nc.vector.reciprocal(recip[:], num_ps[:, DEPTH:JAUG])
                        nc.scalar.activation(attn2[:, jo:jo + DEPTH],
                                             num_ps[:, 0:DEPTH], AF.Copy,
                                             scale=recip[:])

                        # state delta: S_aug_h += K_h^T V_aug_h
                        if ci < NCH - 1:
                            mm(d_ps[jo:jo + DEPTH, :],
                               knat_sb[:, ci, h, :], vt,
                               start=True, stop=True)

                    # state update for both heads of this jb at once
                    if ci < NCH - 1:
                        if ci == 0:
                            nc.vector.tensor_copy(saug_sb[jb][:], d_ps[:])
                        else:
                            nc.vector.tensor_add(saug_sb[jb][:],
                                                 saug_sb[jb][:], d_ps[:])
                        nc.scalar.activation(saug_bf[jb][:], saug_sb[jb][:],
                                             AF.Copy)

                    # attn^T for both heads via DMA-xbar transpose
                    nc.sync.dma_start_transpose(
                        aT_sb[jb][:, scol:scol + C], attn2[:])

            # ---- Phase 3: output projection (partial: this core's heads) ---
            po_r = po.rearrange("(ob p) s -> ob p s", p=128)
            OSC = 512
            for ob in range(D // 128):
                for sq in range(S // OSC):
                    ps = psA.tile([128, OSC], f32, tag="A")
                    for jb in range(2):
                        mm(ps[:], wo_sb[:, jb, ob * 128:(ob + 1) * 128],
                           aT_sb[jb][:, sq * OSC:(sq + 1) * OSC],
                           start=(jb == 0), stop=(jb == 1))
                    ot = opool.tile([128, OSC], bf16, tag="ot")
                    if (ob + sq) % 2 == 0:
                        nc.scalar.activation(ot[:], ps[:], AF.Copy)
                    else:
                        nc.vector.tensor_copy(ot[:], ps[:])
                    nc.sync.dma_start(po_r[ob, :, sq * OSC:(sq + 1) * OSC],
                                      ot[:])

    nc.compile()
    return nc


def _get_prog():
    global _PROG
    if _PROG is None:
        _PROG = _build()
    return _PROG


def kernel(q, k, v, query_mask, key_mask, value_mask,
           Wq, bq, Wk, bk, Wv, bv, Wo, bo):
    global LAST_RESULTS
    from concourse import bass_utils

    q = np.asarray(q, np.float32)
    k = np.asarray(k, np.float32)
    v = np.asarray(v, np.float32)
    qm = q * np.asarray(query_mask, np.float32)
    km = k * np.asarray(key_mask, np.float32)
    vm = v * np.asarray(value_mask, np.float32)
    Wq = np.asarray(Wq, np.float32)
    Wk = np.asarray(Wk, np.float32)
    Wv = np.asarray(Wv, np.float32)
    Wo = np.asarray(Wo, np.float32)
    bq = np.asarray(bq, np.float32)
    bk = np.asarray(bk, np.float32)
    bv = np.asarray(bv, np.float32)
    bo = np.asarray(bo, np.float32)
    assert not np.any(bv), "kernel assumes bv == 0 (true for this problem)"

    nc = _get_prog()

    triu = np.triu(np.ones((128, 128), np.float32))
    xqs = [np.ascontiguousarray(qm[b].T).astype(BF16) for b in range(B)]
    xks = [np.ascontiguousarray(km[b].T).astype(BF16) for b in range(B)]
    xvs = [np.ascontiguousarray(vm[b].T).astype(BF16) for b in range(B)]

    in_maps = []
    for c in range(N_CORES):
        b, g = divmod(c, HPC)
        js = slice(g * JS, (g + 1) * JS)
        in_maps.append({
            "xq": xqs[b], "xk": xks[b], "xv": xvs[b],
            "wq": np.ascontiguousarray(Wq[:, js]).astype(BF16),
            "wk": np.ascontiguousarray(Wk[:, js]).astype(BF16),
            "wv": np.ascontiguousarray(Wv[:, js]).astype(BF16),
            "wo": np.ascontiguousarray(Wo[js, :]).astype(BF16),
            "bq": np.ascontiguousarray(bq[js].reshape(2, 128)),
            "bk": np.ascontiguousarray(bk[js].reshape(2, 128)),
            "triu": triu,
        })

    res = bass_utils.run_bass_kernel_spmd(
        nc, in_maps, core_ids=list(range(N_CORES)),
        trace=TRACE, trace_cores=TRACE_CORES)
    LAST_RESULTS = res

    out = np.zeros((B, S, D), np.float32)
    for c in range(N_CORES):
        out[c // HPC] += np.asarray(res.results[c]["po"], np.float32).T
    out += bo
    return out


# revision 3
# speedup vs baseline: 2.1603x; 1.5585x over previous
"""Trainium2 Bass kernel for nn_MultiHeadAttention_KT (causal linear attention).

Math (per batch b):
  q' = leaky((q*qm) @ Wq + bq); k' = leaky((k*km) @ Wk + bk); v' = (v*vm) @ Wv
  per head h (DEPTH=64):   S_t = sum_{s<=t} k_s v_s^T ; z_t = sum_{s<=t} k_s
                           attn_t = (q_t @ S_t) / (q_t . z_t)
  out = concat_heads(attn) @ Wo + bo

Sharding: 8 cores = 2 batches x 4 head-groups (4 heads / 256 cols each).
Host transposes inputs (xq = (q*qm)^T etc.), casts everything to bf16,
and sums the 4 partial output projections per batch.

All matmuls run in bf16 (1 cycle/row on PE; f32 accumulate in PSUM).
Transposes (K -> natural layout, attn -> attn^T) run on the DMA xbar in
batched [128, 256..512] -> 3D form, not on the PE. DMA instruction count
is kept low (each dma_start costs ~0.7-1.2us of issuing-engine time):
x loads at 1MB granularity, po stores batched 4 row-blocks per DMA on
the otherwise-idle GpSimd engine.

Chunked linear attention on device (chunk C=128, all matmuls on PE):
  AT   = K Q^T (per chunk, [s,t] layout)      masked with triu (s<=t)
  num  = ATm^T V_aug + Q S_aug                (V_aug = [V | 1], S_aug = [S | z])
  attn = num[:, :64] * (1/num[:, 64])
  S_aug += K_chunk^T V_aug    (delta matmul; f32 master state + bf16 copy)
"""

import os
import sys

sys.path.insert(0, "/opt/trn_rl_repo")

import numpy as np
import ml_dtypes

BF16 = ml_dtypes.bfloat16

B, S, D, H = 2, 2048, 1024, 16
DEPTH = 64
N_CORES = 8
HPC = 4                 # heads per core
JS = HPC * DEPTH        # 256 projected columns per core
C = 128                 # attention chunk length
NCH = S // C            # 16 chunks
IB = D // 128           # 8 contraction blocks
SCH = 512               # projection s-chunk
NSC = S // SCH          # 4 projection chunks
LCH = 1024              # x-input load chunk
JAUG = DEPTH + 1        # 65 (V augmented with ones column)

TRACE = False           # set True from test harness to capture NTFF profile
TRACE_CORES = None
LAST_RESULTS = None     # BassKernelResults of the last kernel() call

_PROG = None


def _build():
    import concourse.bacc as bacc
    import concourse.mybir as mybir
    import concourse.tile as tile

    dt = mybir.dt
    f32 = dt.float32
    bf16 = dt.bfloat16
    AF = mybir.ActivationFunctionType
    Alu = mybir.AluOpType

    nc = bacc.Bacc("TRN2", target_bir_lowering=False, debug=False,
                   num_devices=N_CORES)

    xq = nc.dram_tensor("xq", [D, S], bf16, kind="ExternalInput").ap()
    xk = nc.dram_tensor("xk", [D, S], bf16, kind="ExternalInput").ap()
    xv = nc.dram_tensor("xv", [D, S], bf16, kind="ExternalInput").ap()
    wq = nc.dram_tensor("wq", [D, JS], bf16, kind="ExternalInput").ap()
    wk = nc.dram_tensor("wk", [D, JS], bf16, kind="ExternalInput").ap()
    wv = nc.dram_tensor("wv", [D, JS], bf16, kind="ExternalInput").ap()
    wo = nc.dram_tensor("wo", [JS, D], bf16, kind="ExternalInput").ap()
    bqd = nc.dram_tensor("bq", [2, 128], f32, kind="ExternalInput").ap()
    bkd = nc.dram_tensor("bk", [2, 128], f32, kind="ExternalInput").ap()
    triu = nc.dram_tensor("triu", [128, 128], f32, kind="ExternalInput").ap()
    po = nc.dram_tensor("po", [D, S], bf16, kind="ExternalOutput").ap()

    def mm(out, lhsT, rhs, **kw):
        nc.tensor.matmul(out, lhsT, rhs, **kw)

    with tile.TileContext(nc) as tc:
        with (
            tc.tile_pool(name="persist", bufs=1) as pp,
            tc.tile_pool(name="xin", bufs=2) as xpool,
            tc.tile_pool(name="work", bufs=4) as wk_pool,
            tc.tile_pool(name="outp", bufs=3) as opool,
            tc.tile_pool(name="psA", bufs=4, space="PSUM") as psA,
            tc.tile_pool(name="psB", bufs=4, space="PSUM") as psB,
        ):
            # ---- Phase 0: weights + constants -------------------------------
            wq_sb = pp.tile([128, IB, JS], bf16, tag="wq", name="wq_sb")
            wk_sb = pp.tile([128, IB, JS], bf16, tag="wk", name="wk_sb")
            wv_sb = pp.tile([128, IB, JS], bf16, tag="wv", name="wv_sb")
            wo_sb = pp.tile([128, 2, D], bf16, tag="wo", name="wo_sb")
            nc.sync.dma_start(wq_sb[:], wq.rearrange("(ib p) j -> p ib j", p=128))
            nc.sync.dma_start(wk_sb[:], wk.rearrange("(ib p) j -> p ib j", p=128))
            nc.sync.dma_start(wv_sb[:], wv.rearrange("(ib p) j -> p ib j", p=128))
            nc.sync.dma_start(wo_sb[:], wo.rearrange("(jb p) o -> p jb o", p=128))
            bq_sb = pp.tile([128, 2], f32, tag="bq", name="bq_sb")
            bk_sb = pp.tile([128, 2], f32, tag="bk", name="bk_sb")
            nc.sync.dma_start(bq_sb[:], bqd.rearrange("jb p -> p jb"))
            nc.sync.dma_start(bk_sb[:], bkd.rearrange("jb p -> p jb"))
            triu_sb = pp.tile([128, 128], f32, tag="triu", name="triu_sb")
            nc.sync.dma_start(triu_sb[:], triu)

            qT_sb = [pp.tile([128, S], bf16, tag=f"qT{jb}", name=f"qT{jb}") for jb in range(2)]
            kT_sb = [pp.tile([128, S], bf16, tag=f"kT{jb}", name=f"kT{jb}") for jb in range(2)]
            # attn^T, [128 j, jb, s]
            aT_sb = pp.tile([128, 2, S], bf16, tag="aT", name="aT_sb")
            # K natural per jb: [128 s%128, chunk, 128 j] (2 heads along j)
            knat_sb = [pp.tile([128, NCH, 128], bf16, tag=f"knat{jb}",
                               name=f"knat{jb}")
                       for jb in range(2)]
            vaug_sb = [pp.tile([128, HPC * JAUG], bf16, tag=f"vaug{i}",
                               name=f"vaug{i}")
                       for i in range(NCH)]
            # two heads per tile: head h at partitions (h%2)*64 .. +64
            saug_sb = [pp.tile([128, JAUG], f32, tag=f"saug{jb}", name=f"saug{jb}")
                       for jb in range(2)]
            saug_bf = [pp.tile([128, JAUG], bf16, tag=f"saugb{jb}",
                               name=f"saugb{jb}")
                       for jb in range(2)]

            xq_r = xq.rearrange("(ib p) s -> p ib s", p=128)
            xk_r = xk.rearrange("(ib p) s -> p ib s", p=128)
            xv_r = xv.rearrange("(ib p) s -> p ib s", p=128)

            # ---- Phase 1: projections --------------------------------------
            for lc in range(S // LCH):
                l0 = lc * LCH
                xq_t = xpool.tile([128, IB, LCH], bf16, tag="xq")
                xk_t = xpool.tile([128, IB, LCH], bf16, tag="xk")
                xv_t = xpool.tile([128, IB, LCH], bf16, tag="xv")
                nc.sync.dma_start(xq_t[:], xq_r[:, :, l0:l0 + LCH])
                nc.sync.dma_start(xk_t[:], xk_r[:, :, l0:l0 + LCH])
                nc.sync.dma_start(xv_t[:], xv_r[:, :, l0:l0 + LCH])

                for scc in range(LCH // SCH):
                    sc = lc * (LCH // SCH) + scc
                    s0 = sc * SCH
                    c0 = scc * SCH

                    # q'/k' transposed: psum [128 j, SCH s]
                    for name, w_sb, x_t, b_sb, dst in (
                        ("q", wq_sb, xq_t, bq_sb, qT_sb),
                        ("k", wk_sb, xk_t, bk_sb, kT_sb),
                    ):
                        for jb in range(2):
                            ps = psA.tile([128, SCH], f32, tag="A")
                            for ib in range(IB):
                                mm(ps[:], w_sb[:, ib, jb * 128:(jb + 1) * 128],
                                   x_t[:, ib, c0:c0 + SCH],
                                   start=(ib == 0), stop=(ib == IB - 1))
                            nc.scalar.activation(
                                dst[jb][:, s0:s0 + SCH], ps[:], AF.Prelu,
                                bias=b_sb[:, jb:jb + 1], scale=1.0, alpha=0.1)

                    # v' natural: psum [128 s, JS], augmented store
                    for ss in range(SCH // 128):
                        ps = psA.tile([128, JS], f32, tag="A")
                        for ib in range(IB):
                            mm(ps[:], xv_t[:, ib, c0 + ss * 128:c0 + (ss + 1) * 128],
                               wv_sb[:, ib, :],
                               start=(ib == 0), stop=(ib == IB - 1))
                        vt = vaug_sb[sc * (SCH // 128) + ss]
                        vt_r = vt[:].rearrange("p (h e) -> p h e", h=HPC)
                        nc.scalar.activation(
                            vt_r[:, :, 0:DEPTH],
                            ps[:].rearrange("p (h e) -> p h e", h=HPC), AF.Copy)
                        nc.gpsimd.memset(vt_r[:, :, DEPTH:JAUG], 1.0)

                    # K natural for these 4 chunks (both heads of each jb
                    # at once) via one batched DMA-xbar transpose per jb
                    for jb in range(2):
                        nc.sync.dma_start_transpose(
                            knat_sb[jb][:, sc * 4:(sc + 1) * 4, :],
                            kT_sb[jb][:, s0:s0 + SCH])

            # ---- Phase 2: chunked causal linear attention ------------------
            for ci in range(NCH):
                scol = ci * C
                attn2 = wk_pool.tile([128, 2, 2 * DEPTH], bf16, tag="attn2")
                for jb in range(2):
                    d_ps = psB.tile([128, JAUG], f32, tag="B")
                    for hh in range(2):
                        h = jb * 2 + hh
                        jo = hh * DEPTH
                        kT_v = kT_sb[jb][jo:jo + DEPTH, scol:scol + C]
                        qT_v = qT_sb[jb][jo:jo + DEPTH, scol:scol + C]
                        vt = vaug_sb[ci][:, h * JAUG:(h + 1) * JAUG]

                        # AT = K Q^T  [s, t]; mask s<=t
                        at_ps = psA.tile([128, C], f32, tag="A")
                        mm(at_ps[:], kT_v, qT_v, start=True, stop=True)
                        atm = wk_pool.tile([128, C], bf16, tag="atm")
                        nc.vector.tensor_tensor(atm[:], at_ps[:], triu_sb[:],
                                                op=Alu.mult)

                        # num[t, 0:64] + den[t, 64]
                        num_ps = psB.tile([128, JAUG], f32, tag="B")
                        mm(num_ps[:], atm[:], vt, start=True, stop=(ci == 0))
                        if ci > 0:
                            mm(num_ps[:], qT_v,
                               saug_bf[jb][jo:jo + DEPTH, :],
                               start=False, stop=True)

                        recip = wk_pool.tile([128, 1], f32, tag="recip")
                        nc.vector.reciprocal(recip[:], num_ps[:, DEPTH:JAUG])
                        nc.scalar.activation(attn2[:, jb, jo:jo + DEPTH],
                                             num_ps[:, 0:DEPTH], AF.Copy,
                                             scale=recip[:])

                        # state delta: S_aug_h += K_h^T V_aug_h
                        if ci < NCH - 1:
                            mm(d_ps[jo:jo + DEPTH, :],
                               knat_sb[jb][:, ci, jo:jo + DEPTH], vt,
                               start=True, stop=True)

                    # state update for both heads of this jb at once
                    if ci < NCH - 1:
                        if ci == 0:
                            nc.vector.tensor_copy(saug_sb[jb][:], d_ps[:])
                        else:
                            nc.vector.tensor_add(saug_sb[jb][:],
                                                 saug_sb[jb][:], d_ps[:])
                        nc.scalar.activation(saug_bf[jb][:], saug_sb[jb][:],
                                             AF.Copy)

                # attn^T for all 4 heads in one batched xbar transpose
                nc.sync.dma_start_transpose(
                    aT_sb[:, :, scol:scol + C],
                    attn2[:].rearrange("p a b -> p (a b)"))

            # ---- Phase 3: output projection (partial: this core's heads) ---
            po_r = po.rearrange("(g ob p) s -> g p ob s", ob=4, p=128)
            OSC = 512
            for g in range(2):
                for sq in range(S // OSC):
                    ot = opool.tile([128, 4, OSC], bf16, tag="ot")
                    for oo in range(4):
                        ob = g * 4 + oo
                        ps = psA.tile([128, OSC], f32, tag="A")
                        for jb in range(2):
                            mm(ps[:], wo_sb[:, jb, ob * 128:(ob + 1) * 128],
                               aT_sb[:, jb, sq * OSC:(sq + 1) * OSC],
                               start=(jb == 0), stop=(jb == 1))
                        if oo % 2 == 0:
                            nc.scalar.activation(ot[:, oo, :], ps[:], AF.Copy)
                        else:
                            nc.vector.tensor_copy(ot[:, oo, :], ps[:])
                    nc.gpsimd.dma_start(
                        po_r[g, :, :, sq * OSC:(sq + 1) * OSC], ot[:])

    nc.compile()
    return nc


def _get_prog():
    global _PROG
    if _PROG is None:
        _PROG = _build()
    return _PROG


def kernel(q, k, v, query_mask, key_mask, value_mask,
           Wq, bq, Wk, bk, Wv, bv, Wo, bo):
    global LAST_RESULTS
    from concourse import bass_utils

    q = np.asarray(q, np.float32)
    k = np.asarray(k, np.float32)
    v = np.asarray(v, np.float32)
    qm = q * np.asarray(query_mask, np.float32)
    km = k * np.asarray(key_mask, np.float32)
    vm = v * np.asarray(value_mask, np.float32)
    Wq = np.asarray(Wq, np.float32)
    Wk = np.asarray(Wk, np.float32)
    Wv = np.asarray(Wv, np.float32)
    Wo = np.asarray(Wo, np.float32)
    bq = np.asarray(bq, np.float32)
    bk = np.asarray(bk, np.float32)
    bv = np.asarray(bv, np.float32)
    bo = np.asarray(bo, np.float32)
    assert not np.any(bv), "kernel assumes bv == 0 (true for this problem)"

    nc = _get_prog()

    triu = np.triu(np.ones((128, 128), np.float32))
    xqs = [np.ascontiguousarray(qm[b].T).astype(BF16) for b in range(B)]
    xks = [np.ascontiguousarray(km[b].T).astype(BF16) for b in range(B)]
    xvs = [np.ascontiguousarray(vm[b].T).astype(BF16) for b in range(B)]

    in_maps = []
    for c in range(N_CORES):
        b, g = divmod(c, HPC)
        js = slice(g * JS, (g + 1) * JS)
        in_maps.append({
            "xq": xqs[b], "xk": xks[b], "xv": xvs[b],
            "wq": np.ascontiguousarray(Wq[:, js]).astype(BF16),
            "wk": np.ascontiguousarray(Wk[:, js]).astype(BF16),
            "wv": np.ascontiguousarray(Wv[:, js]).astype(BF16),
            "wo": np.ascontiguousarray(Wo[js, :]).astype(BF16),
            "bq": np.ascontiguousarray(bq[js].reshape(2, 128)),
            "bk": np.ascontiguousarray(bk[js].reshape(2, 128)),
            "triu": triu,
        })

    res = bass_utils.run_bass_kernel_spmd(
        nc, in_maps, core_ids=list(range(N_CORES)),
        trace=TRACE, trace_cores=TRACE_CORES)
    LAST_RESULTS = res

    out = np.zeros((B, S, D), np.float32)
    for c in range(N_CORES):
        out[c // HPC] += np.asarray(res.results[c]["po"], np.float32).T
    out += bo
    return out
